# revision 7
# baseline (speedup 1.0000x reference)
"""TransformerConv GNN message passing on 8 TRN2 NeuronCores (Bass/Tile).

v4 strategy (dst-sharded edge parallelism, two launches):
  - Core c owns 6250 destination nodes; edges are sharded by dst so
    segment-softmax and scatter-aggregation stay core-local (no collectives).
  - Launch A computes q = x @ Wq (+bq) for each core's own nodes; the host
    gathers q[dst] per edge (host-side gather per the sharding hint).
  - Host packs per-core fp8/bf16 streams, 640B/edge (vs 1024B baseline):
      xs8 = fp8(x[src])^T, ea8 = fp8(edge_attr)^T   (features on partitions)
      oh8 = fp8 onehot(dst_local within window)     (edges on partitions)
      qd16 = bf16 q[dst]                            (edges on partitions)
  - Launch B, per 128-edge subchunk:
      kve = [xs8;ea8] @ [[Wk|Wv];[We|We]]  -- ONE fp8 DoubleRow matmul
            (256-wide contraction in a single pass, f32 PSUM)
      cum = MULSCAN(kve_k, qd)             -- custom DVE uop: running
            cumsum of the q.k product; per-(subchunk,head) dots fall out
            as differences of segment-end cumsums (fuses mul+reduce)
      pe  = exp(alpha/8) on ACT (tiny [128, G*2], not broadcast)
      ve  = kve_v * pe (DVE, PSUM-direct)
      agg[128,130] += onehot^T @ [ve | pe] -- PE scatter, PSUM-accumulated
    Window epilogue: out = (agg/denom) @ Wproj + x_win @ (Wskip@Wproj).
  - Softmax max-shift dropped (alpha/8 is O(1); mathematically identical),
    normalization applied post-aggregation (linearity).
  - Windows are sorted by edge count per core so slot k holds each core's
    k-th largest window: the shared SPMD schedule S[k] = max_c count then
    wastes minimal padding.

kernel(**inputs) takes FULL unsharded inputs, returns the FULL [50000,128]
f32 output.  TRACE=True captures NTFF timing (LAST_EXEC_TIME_NS = sum of
both launches; LAST_RESULTS = launch-B results).
"""
import sys
from contextlib import ExitStack

import numpy as np

for _p in ('/opt/trn_rl_repo', '/root/.axon_site/_ro/trn_rl_repo'):
    if _p not in sys.path:
        sys.path.append(_p)

import ml_dtypes

import concourse.bass as bass          # noqa: E402
import concourse.mybir as mybir        # noqa: E402
import concourse.tile as tile          # noqa: E402
from concourse import bacc             # noqa: E402
from concourse import bass_utils       # noqa: E402

bf16 = ml_dtypes.bfloat16
f8 = ml_dtypes.float8_e4m3fn
F32 = mybir.dt.float32
BF16 = mybir.dt.bfloat16
FP8 = mybir.dt.float8e4

N = 50000
E = 800000
DIM = 128
H = 2
C = 64
P = 128
NCORES = 8
NODES_PER_CORE = N // NCORES          # 6250
WIN = 128
NWIN = (NODES_PER_CORE + WIN - 1) // WIN   # 49
NODES_PAD = NWIN * WIN                # 6272
GROUP = 4
ALPHA_SCALE = 0.125                   # 1/sqrt(64)
SUBB = 5 * P                          # stream bytes/partition per subchunk

TRACE = False
LAST_EXEC_TIME_NS = None
LAST_RESULTS = None

# ---------------------------------------------------------------------------
# custom DVE op: out = cumsum(in0 * in1) along the free dim (f32 scan)
# ---------------------------------------------------------------------------
import concourse.dve_ops as dve_ops                      # noqa: E402
from concourse.dve_ops import DveOp, OPS                 # noqa: E402
from concourse.dve_spec import Spec, Src0, Src1, AluOp, lower, scan  # noqa: E402
from concourse.dve_uop import DveOpSpec                  # noqa: E402


def _ref_mulscan(in0, in1, s0, s1, imm2):
    prod = in0.astype(np.float32) * in1.astype(np.float32)
    return np.cumsum(prod.reshape(prod.shape[0], -1), axis=1).reshape(prod.shape)


def _register_mulscan():
    if "MULSCAN_ANT" in dve_ops._SUB_OPCODE_FOR_NAME:
        return next(op for op in OPS if op.name == "MULSCAN_ANT")
    spec = Spec(body=scan(AluOp.ADD, Src0 * Src1), reference=_ref_mulscan)
    shas = {}
    for ver in ("v3", "v4"):
        sp = DveOpSpec(name="MULSCAN_ANT", opcode=31,
                       uops=lower(spec, ver=ver), rd1_en=True)
        shas[ver] = sp.sha(ver)
    op = DveOp("MULSCAN_ANT", spec, subdim=False, uops_sha=shas)
    OPS.append(op)
    dve_ops._SUB_OPCODE_FOR_NAME["MULSCAN_ANT"] = (
        max(dve_ops._SUB_OPCODE_FOR_NAME.values()) + 1)
    return op


MULSCAN = _register_mulscan()


# ----------------------------------------------------------------------------
# host-side sharding / preprocessing
# ----------------------------------------------------------------------------

def _schedule(S):
    """Groups of <=GROUP subchunks sharing one DMA; off in stream columns."""
    groups = []
    off = 0
    sub_base = 0
    for w in range(NWIN):
        for g0 in range(0, S[w], GROUP):
            Wg = min(GROUP, S[w] - g0)
            groups.append((w, sub_base + g0, Wg, off))
            off += Wg * SUBB
        sub_base += S[w]
    return groups, off


def _shard(edge_index):
    """Dst-shard edges; sort windows per core by count for minimal padding."""
    src = np.asarray(edge_index[0], dtype=np.int64)
    dst = np.asarray(edge_index[1], dtype=np.int64)
    core_of = dst // NODES_PER_CORE
    dst_local = dst - core_of * NODES_PER_CORE
    win_of = dst_local // WIN

    counts = np.zeros((NCORES, NWIN), dtype=np.int64)
    np.add.at(counts, (core_of, win_of), 1)

    # slot k on every core holds that core's k-th largest window
    win_perm = np.argsort(-counts, axis=1, kind='stable')   # [core, slot]->win
    sorted_counts = np.take_along_axis(counts, win_perm, axis=1)
    S = np.maximum(np.ceil(sorted_counts / P).astype(np.int64).max(axis=0), 1)
    TS = int(S.sum())
    EPAD = TS * P

    order = np.lexsort((np.arange(E), win_of, core_of))
    run_ends = np.cumsum(counts.reshape(-1))
    run_starts = np.concatenate([[0], run_ends[:-1]]).reshape(NCORES, NWIN)
    run_ends = run_ends.reshape(NCORES, NWIN)
    wbase = np.concatenate([[0], np.cumsum(S)])
    return dict(src=src, dst=dst, dst_local=dst_local, win_perm=win_perm,
                S=S.tolist(), TS=TS, EPAD=EPAD, order=order,
                run_starts=run_starts, run_ends=run_ends, wbase=wbase)


def _pack_streams(x, edge_attr, q_all, sh):
    """Per-core packed stream [128, total_cols] fp8-bytes, and xTown."""
    x_np = np.asarray(x, dtype=np.float32)
    ea_np = np.asarray(edge_attr, dtype=np.float32)
    S, TS, EPAD = sh['S'], sh['TS'], sh['EPAD']
    groups, total_cols = _schedule(S)
    x8 = x_np.astype(f8)

    per_core = []
    xtowns = []
    for c in range(NCORES):
        src_pad = np.zeros(EPAD, dtype=np.int64)
        dstoh_pad = np.full(EPAD, -1, dtype=np.int64)
        ea_rows = np.zeros(EPAD, dtype=np.int64)
        ea_valid = np.zeros(EPAD, dtype=bool)
        qd_rows = np.zeros(EPAD, dtype=np.int64)
        for k in range(NWIN):
            w = int(sh['win_perm'][c, k])
            sel = sh['order'][sh['run_starts'][c, w]:sh['run_ends'][c, w]]
            cnt = len(sel)
            base = int(sh['wbase'][k]) * P
            src_pad[base:base + cnt] = sh['src'][sel]
            dstoh_pad[base:base + cnt] = sh['dst_local'][sel] - w * WIN
            ea_rows[base:base + cnt] = sel
            ea_valid[base:base + cnt] = True
            qd_rows[base:base + cnt] = sh['dst_local'][sel]

        ea8 = np.zeros((EPAD, DIM), dtype=f8)
        ea8[ea_valid] = ea_np[ea_rows[ea_valid]].astype(f8)
        xs8 = x8[src_pad]
        xs8[~ea_valid] = 0
        oh8 = np.zeros((EPAD, P), dtype=f8)
        vmask = dstoh_pad >= 0
        oh8[np.nonzero(vmask)[0], dstoh_pad[vmask]] = 1.0
        qd16 = q_all[c][qd_rows]                       # [EPAD,128] bf16
        qd16[~ea_valid] = 0

        def sub_t(mat):   # feature dim on partitions, per 128-edge sub-chunk
            return np.ascontiguousarray(
                mat.reshape(TS, P, P).transpose(2, 0, 1).reshape(P, EPAD))

        def sub_n(mat):   # edges on partitions
            return np.ascontiguousarray(
                mat.reshape(TS, P, -1).transpose(1, 0, 2).reshape(P, -1))

        xs_s = sub_t(xs8).view(np.uint8)
        ea_s = sub_t(ea8).view(np.uint8)
        oh_s = sub_n(oh8).view(np.uint8)
        qd_s = sub_n(qd16).view(np.uint8).reshape(P, EPAD * 2)

        edge_pm = np.empty((P, total_cols), dtype=np.uint8)
        for (_w, s0, Wg, off) in groups:
            W128 = Wg * P
            e0 = s0 * P
            blk = edge_pm[:, off:off + Wg * SUBB]
            blk[:, 0:W128] = xs_s[:, e0:e0 + W128]
            blk[:, W128:2 * W128] = ea_s[:, e0:e0 + W128]
            blk[:, 2 * W128:3 * W128] = oh_s[:, e0:e0 + W128]
            blk[:, 3 * W128:5 * W128] = qd_s[:, 2 * e0:2 * (e0 + W128)]
        per_core.append(edge_pm.view(f8))

        own = np.zeros((NODES_PAD, DIM), dtype=np.float32)
        own[:NODES_PER_CORE] = x_np[c * NODES_PER_CORE:(c + 1) * NODES_PER_CORE]
        own_perm = own.reshape(NWIN, WIN, DIM)[sh['win_perm'][c]]
        xtowns.append(np.ascontiguousarray(
            own_perm.reshape(NODES_PAD, DIM).T).astype(bf16))

    return per_core, xtowns, groups, total_cols


# ----------------------------------------------------------------------------
# launch A: q = x @ Wq (+ bq) for own nodes
# ----------------------------------------------------------------------------

def _build_q(has_bias):
    nc = bacc.Bacc("TRN2", target_bir_lowering=False, debug=False)
    xT_in = nc.dram_tensor("xTown_pm", [P, NODES_PAD], BF16,
                           kind="ExternalInput").ap()
    wq_in = nc.dram_tensor("wq", [P, P], F32, kind="ExternalInput").ap()
    if has_bias:
        bq_in = nc.dram_tensor("bq_row", [1, P], F32, kind="ExternalInput").ap()
    q_out = nc.dram_tensor("q_out", [NODES_PAD, P], BF16,
                           kind="ExternalOutput").ap()

    with tile.TileContext(nc) as tc, ExitStack() as top:
        res = top.enter_context(tc.tile_pool(name="res", bufs=1))
        xT = res.tile([P, NODES_PAD], BF16)
        nc.sync.dma_start(out=xT[:], in_=xT_in[:, :])
        wqf = res.tile([P, P], F32)
        nc.sync.dma_start(out=wqf[:], in_=wq_in[:, :])
        wq16 = res.tile([P, P], BF16)
        nc.vector.tensor_copy(out=wq16[:], in_=wqf[:])
        if has_bias:
            bqf = res.tile([1, P], F32)
            nc.sync.dma_start(out=bqf[:], in_=bq_in[:, :])
            bq16 = res.tile([1, P], BF16)
            nc.vector.tensor_copy(out=bq16[:], in_=bqf[:])
            ones_row = res.tile([1, P], BF16)
            nc.vector.memset(ones_row[:], 1.0)

        with tc.tile_pool(name="qp", bufs=4) as qp, \
             tc.tile_pool(name="qps", bufs=4, space="PSUM") as qps:
            for w in range(NWIN):
                q_ps = qps.tile([P, P], F32, tag="q")
                nc.tensor.matmul(out=q_ps[:], lhsT=xT[:, w * P:(w + 1) * P],
                                 rhs=wq16[:], start=True,
                                 stop=not has_bias, skip_group_check=True)
                if has_bias:
                    nc.tensor.matmul(out=q_ps[:], lhsT=ones_row[:],
                                     rhs=bq16[:], start=False, stop=True,
                                     skip_group_check=True)
                q_sb = qp.tile([P, P], BF16, tag="qsb")
                nc.scalar.copy(out=q_sb[:], in_=q_ps[:])
                nc.sync.dma_start(out=q_out[w * P:(w + 1) * P, :], in_=q_sb[:])
    nc.compile()
    return nc


# ----------------------------------------------------------------------------
# launch B: main edge kernel
# ----------------------------------------------------------------------------

def _build_main(S, groups, total_cols, has_bias):
    nc = bacc.Bacc("TRN2", target_bir_lowering=False, debug=False)

    edge_pm = nc.dram_tensor("edge_pm", [P, total_cols], FP8,
                             kind="ExternalInput").ap()
    xTown_pm = nc.dram_tensor("xTown_pm", [P, NODES_PAD], BF16,
                              kind="ExternalInput").ap()
    ident_in = nc.dram_tensor("ident_in", [P, P], BF16,
                              kind="ExternalInput").ap()
    w_in = {}
    for name in ["wk", "wv", "we", "wskip", "wproj"]:
        w_in[name] = nc.dram_tensor(name, [P, P], F32, kind="ExternalInput").ap()
    if has_bias:
        bkv_row = nc.dram_tensor("bkv_row", [1, 2 * P], F32,
                                 kind="ExternalInput").ap()
        bskip_col = nc.dram_tensor("bskip_col", [P, 1], F32,
                                   kind="ExternalInput").ap()
        bproj_row = nc.dram_tensor("bproj_row", [1, P], F32,
                                   kind="ExternalInput").ap()
    out = nc.dram_tensor("out", [NODES_PAD, DIM], F32, kind="ExternalOutput").ap()

    with tile.TileContext(nc) as tc, ExitStack() as top:
        res = top.enter_context(tc.tile_pool(name="res", bufs=1))

        xTown_sb = res.tile([P, NODES_PAD], BF16)
        nc.sync.dma_start(out=xTown_sb[:], in_=xTown_pm[:, :])
        ident = res.tile([P, P], BF16)
        nc.sync.dma_start(out=ident[:], in_=ident_in[:, :])

        wsb = {}
        for name in ["wk", "wv", "we", "wskip", "wproj"]:
            wf = res.tile([P, P], F32, tag="wf32")
            nc.sync.dma_start(out=wf[:], in_=w_in[name][:, :])
            wb = res.tile([P, P], BF16, tag=f"{name}_b")
            nc.vector.tensor_copy(out=wb[:], in_=wf[:])
            wsb[name] = wb
        # wstack8: [P, 2, 256] fp8 = [[Wk|Wv] ; [We|We]]
        wstack8 = res.tile([P, 2, 2 * P], FP8)
        nc.vector.tensor_copy(out=wstack8[:, 0, 0:P], in_=wsb["wk"][:])
        nc.vector.tensor_copy(out=wstack8[:, 0, P:2 * P], in_=wsb["wv"][:])
        nc.vector.tensor_copy(out=wstack8[:, 1, 0:P], in_=wsb["we"][:])
        nc.vector.tensor_copy(out=wstack8[:, 1, P:2 * P], in_=wsb["we"][:])

        if has_bias:
            bkv_sb = res.tile([1, 2 * P], BF16)
            ones_row = res.tile([1, P], BF16)
            nc.vector.memset(ones_row[:], 1.0)
            bkvf = res.tile([1, 2 * P], F32)
            nc.sync.dma_start(out=bkvf[:], in_=bkv_row[:, :])
            nc.vector.tensor_copy(out=bkv_sb[:], in_=bkvf[:])
            bskipc = res.tile([P, 1], F32)
            nc.sync.dma_start(out=bskipc[:], in_=bskip_col[:, :])
            bskipc_b = res.tile([P, 1], BF16)
            nc.vector.tensor_copy(out=bskipc_b[:], in_=bskipc[:])
            bprojf = res.tile([1, P], F32)
            nc.sync.dma_start(out=bprojf[:], in_=bproj_row[:, :])

        # fused skip weight: Wfused = Wskip @ Wproj (and fused bias)
        wfused_sb = res.tile([P, P], BF16)
        bfused_sb = res.tile([1, P], BF16, name="bfused_sb") if has_bias else None
        with tc.tile_pool(name="wset_ps", bufs=1, space="PSUM") as wps_pool, \
             tc.tile_pool(name="wset_sb", bufs=1) as wsb_pool:
            tp = wps_pool.tile([P, P], BF16)
            nc.tensor.transpose(out=tp[:], in_=wsb["wskip"][:], identity=ident[:])
            wskipT = wsb_pool.tile([P, P], BF16)
            nc.vector.tensor_copy(out=wskipT[:], in_=tp[:])
            wf_ps = wps_pool.tile([P, P], F32)
            nc.tensor.matmul(out=wf_ps[:], lhsT=wskipT[:], rhs=wsb["wproj"][:],
                             start=True, stop=True)
            nc.vector.tensor_copy(out=wfused_sb[:], in_=wf_ps[:])
            if has_bias:
                bf_ps = wps_pool.tile([1, P], F32)
                nc.tensor.matmul(out=bf_ps[:], lhsT=bskipc_b[:],
                                 rhs=wsb["wproj"][:], start=True, stop=True)
                bff = wsb_pool.tile([1, P], F32)
                nc.vector.tensor_add(out=bff[:], in0=bf_ps[:], in1=bprojf[:])
                nc.vector.tensor_copy(out=bfused_sb[:], in_=bff[:])

        # ---------------- main loop -------------
        with tc.tile_pool(name="edge_in", bufs=10) as in_pool, \
             tc.tile_pool(name="work", bufs=6) as wk_pool, \
             tc.tile_pool(name="kve_ps", bufs=2, space="PSUM") as kve_pool, \
             tc.tile_pool(name="agg_ps", bufs=2, space="PSUM") as agg_pool, \
             tc.tile_pool(name="epi_ps", bufs=1, space="PSUM") as epi_pool, \
             tc.tile_pool(name="outp", bufs=6) as out_pool:
            aggs = {}

            def epilogue(k):
                agg = aggs.pop(k)
                den = out_pool.tile([P, H], F32, tag="den", name=f"den{k}")
                nc.vector.tensor_scalar_add(den[:], agg[:, P:P + H], 1e-30)
                inv = out_pool.tile([P, H], F32, tag="inv", name=f"inv{k}")
                nc.vector.reciprocal(out=inv[:], in_=den[:])
                aggn = out_pool.tile([P, P], BF16, tag="aggn", name=f"aggn{k}")
                nc.vector.tensor_mul(
                    out=aggn[:].rearrange("p (h c) -> p h c", c=C),
                    in0=agg[:, 0:P].rearrange("p (h c) -> p h c", c=C),
                    in1=inv[:].unsqueeze(2).broadcast_to([P, H, C]))
                tp_ps = epi_pool.tile([P, P], BF16, tag="tp", name=f"tp{k}")
                nc.tensor.transpose(out=tp_ps[:], in_=aggn[:], identity=ident[:])
                aggT = out_pool.tile([P, P], BF16, tag="aggT", name=f"aggT{k}")
                nc.scalar.copy(out=aggT[:], in_=tp_ps[:])
                fin = epi_pool.tile([P, P], F32, tag="fin", name=f"fin{k}")
                nc.tensor.matmul(out=fin[:], lhsT=aggT[:], rhs=wsb["wproj"][:],
                                 start=True, stop=False, skip_group_check=True)
                nc.tensor.matmul(out=fin[:], lhsT=xTown_sb[:, k * P:(k + 1) * P],
                                 rhs=wfused_sb[:], start=False,
                                 stop=not has_bias, skip_group_check=True)
                if has_bias:
                    nc.tensor.matmul(out=fin[:], lhsT=ones_row[:],
                                     rhs=bfused_sb[:], start=False, stop=True,
                                     skip_group_check=True)
                fin_sb = out_pool.tile([P, P], F32, tag="fin_sb", name=f"fsb{k}")
                nc.scalar.copy(out=fin_sb[:], in_=fin[:])
                nc.sync.dma_start(out=out[k * P:(k + 1) * P, :], in_=fin_sb[:])

            pend_blk = [None]

            def fetch(gi):
                """DMA two consecutive groups at once; return this group's view."""
                if pend_blk[0] is not None:
                    view = pend_blk[0]
                    pend_blk[0] = None
                    return view
                _, _, Wg0, off0 = groups[gi]
                len0 = Wg0 * SUBB
                if gi + 1 < len(groups):
                    _, _, Wg1, off1 = groups[gi + 1]
                    len1 = Wg1 * SUBB
                else:
                    len1 = 0
                t = in_pool.tile([P, len0 + len1], FP8, tag="blk")
                nc.sync.dma_start(out=t[:], in_=edge_pm[:, off0:off0 + len0 + len1])
                if len1:
                    pend_blk[0] = t[:, len0:len0 + len1]
                return t[:, 0:len0]

            for gi, (k, s0, Wg, off) in enumerate(groups):
                kstart = sum(S[:k])
                Sw = S[k]
                if k not in aggs:
                    aggs[k] = agg_pool.tile([P, P + H], F32, tag="agg",
                                            name=f"agg{k}")
                agg = aggs[k]
                W128 = Wg * P

                blk = fetch(gi)
                xsea = blk[:, 0:2 * W128].rearrange("p (t e) -> p t e", t=2)
                oh_in = blk[:, 2 * W128:3 * W128]
                qd_in = blk[:, 3 * W128:5 * W128].bitcast(BF16)

                # PE: fused k|v DoubleRow matmuls (one per subchunk)
                kve = kve_pool.tile([P, Wg, 2 * P], F32, tag="kve")
                for j in range(Wg):
                    nc.tensor.matmul(
                        out=kve[:, j, :],
                        lhsT=xsea[:, :, j * P:(j + 1) * P],
                        rhs=wstack8[:],
                        perf_mode=mybir.MatmulPerfMode.DoubleRow,
                        start=True, stop=not has_bias, skip_group_check=True)
                    if has_bias:
                        nc.tensor.matmul(out=kve[:, j, :], lhsT=ones_row[:],
                                         rhs=bkv_sb[:], start=False, stop=True,
                                         skip_group_check=True)

                # DVE: fused qk-mul + running-dot (custom scan op).
                # cum layout: scan values live at cols [C, C+W128); col C-1
                # is a memset zero so "previous segment end" reads line up.
                cum = wk_pool.tile([P, 2 * C + GROUP * P], F32, tag="cum")
                nc.vector.memset(cum[:, C - 1:C], 0.0)
                nc.vector._custom_dve(
                    MULSCAN,
                    out=cum[:, C:C + W128].rearrange("p (s n) -> p s n", n=P),
                    in0=kve[:, 0:Wg, 0:P],
                    in1=qd_in[:].rearrange("p (s n) -> p s n", n=P))

                # alpha[j,h] = cum[end of (j,h) segment] - cum[prev end]
                alpha = wk_pool.tile([P, Wg, H], BF16, tag="alpha")
                nc.vector.tensor_sub(
                    out=alpha[:].rearrange("p j h -> p (j h)").unsqueeze(2),
                    in0=cum[:, 2 * C - 1:2 * C - 1 + W128].rearrange(
                        "p (s n) -> p s n", n=C)[:, :, 0:1],
                    in1=cum[:, C - 1:C - 1 + W128].rearrange(
                        "p (s n) -> p s n", n=C)[:, :, 0:1])
                # ACT: pe = exp(alpha/8)
                pe = wk_pool.tile([P, Wg, H], BF16, tag="pe")
                nc.scalar.activation(out=pe[:], in_=alpha[:],
                                     func=mybir.ActivationFunctionType.Exp,
                                     scale=ALPHA_SCALE)

                # DVE: ve = kve_v * pe (PSUM-direct, bf16 out); pe rides along
                # in cols 128:130 so ONE 130-col scatter matmul handles both
                # messages and denominators (single PSUM accumulation chain).
                ve = wk_pool.tile([P, Wg, P + H], BF16, tag="ve")
                nc.vector.tensor_mul(
                    out=ve[:, :, 0:P].rearrange("p j (h c) -> p j h c", c=C),
                    in0=kve[:, 0:Wg, P:2 * P].rearrange(
                        "p j (h c) -> p j h c", c=C),
                    in1=pe[:].unsqueeze(3).broadcast_to([P, Wg, H, C]))
                nc.vector.tensor_copy(out=ve[:, :, P:P + H], in_=pe[:])

                # PE: scatter [messages | denominators] in one matmul
                for j in range(Wg):
                    nd = s0 - kstart + j
                    ohj = oh_in[:, j * P:(j + 1) * P]
                    nc.tensor.matmul(out=agg[:], lhsT=ohj, rhs=ve[:, j, :],
                                     start=(nd == 0), stop=(nd == Sw - 1),
                                     skip_group_check=True)
                if s0 - kstart + Wg == Sw:
                    epilogue(k)

    nc.compile()
    return nc


# ----------------------------------------------------------------------------
# entry point
# ----------------------------------------------------------------------------

def kernel(**inputs):
    global LAST_EXEC_TIME_NS, LAST_RESULTS
    assert np.asarray(inputs['x']).shape == (N, DIM)
    assert np.asarray(inputs['edge_index']).shape == (2, E)

    x_np = np.asarray(inputs['x'], dtype=np.float32)
    biases = {kk: np.asarray(inputs[kk], dtype=np.float32)
              for kk in ['bq', 'bk', 'bv', 'bskip', 'bproj']}
    has_bias = any(np.any(b != 0) for b in biases.values())

    sh = _shard(inputs['edge_index'])

    # ---- launch A: q for own nodes ----
    xtowns_plain = []
    for c in range(NCORES):
        own = np.zeros((NODES_PAD, DIM), dtype=np.float32)
        own[:NODES_PER_CORE] = x_np[c * NODES_PER_CORE:(c + 1) * NODES_PER_CORE]
        xtowns_plain.append(np.ascontiguousarray(own.T).astype(bf16))
    nc_q = _build_q(has_bias)
    in_maps_q = []
    for c in range(NCORES):
        m = dict(xTown_pm=xtowns_plain[c],
                 wq=np.asarray(inputs['Wq'], dtype=np.float32))
        if has_bias:
            m['bq_row'] = np.ascontiguousarray(biases['bq'][None, :])
        in_maps_q.append(m)
    res_q = bass_utils.run_bass_kernel_spmd(
        nc_q, in_maps_q, core_ids=list(range(NCORES)), trace=TRACE)
    q_all = [np.asarray(r['q_out']) for r in res_q.results]   # bf16 [NODES_PAD,128]
    t_q = res_q.exec_time_ns

    # ---- host: gather q per edge, pack streams ----
    per_core, xtowns, groups, total_cols = _pack_streams(
        inputs['x'], inputs['edge_attr'], q_all, sh)

    # ---- launch B ----
    nc = _build_main(sh['S'], groups, total_cols, has_bias)
    ident = np.eye(P, dtype=np.float32).astype(bf16)
    in_maps = []
    for c in range(NCORES):
        m = dict(
            edge_pm=per_core[c],
            xTown_pm=xtowns[c],
            ident_in=ident,
            wk=np.asarray(inputs['Wk'], dtype=np.float32),
            wv=np.asarray(inputs['Wv'], dtype=np.float32),
            we=np.asarray(inputs['We'], dtype=np.float32),
            wskip=np.asarray(inputs['Wskip'], dtype=np.float32),
            wproj=np.asarray(inputs['Wproj'], dtype=np.float32),
        )
        if has_bias:
            m['bkv_row'] = np.ascontiguousarray(
                np.concatenate([biases['bk'], biases['bv']])[None, :])
            m['bskip_col'] = np.ascontiguousarray(biases['bskip'][:, None])
            m['bproj_row'] = np.ascontiguousarray(biases['bproj'][None, :])
        in_maps.append(m)

    res = bass_utils.run_bass_kernel_spmd(
        nc, in_maps, core_ids=list(range(NCORES)), trace=TRACE)
    LAST_EXEC_TIME_NS = ((res.exec_time_ns or 0) + (t_q or 0)) or None
    LAST_RESULTS = res

    # ---- unpermute windows, assemble full output ----
    outs = []
    for c in range(NCORES):
        o = np.asarray(res.results[c]['out'])           # [NODES_PAD,128] slotted
        o = o.reshape(NWIN, WIN, DIM)
        unperm = np.empty_like(o)
        unperm[sh['win_perm'][c]] = o
        outs.append(unperm.reshape(NODES_PAD, DIM)[:NODES_PER_CORE])
    return np.ascontiguousarray(
        np.concatenate(outs, axis=0).astype(np.float32))
